# revision 6
# baseline (speedup 1.0000x reference)
import sys
import contextlib

sys.path.insert(0, "/opt/trn_rl_repo")

import numpy as np
import ml_dtypes

import concourse.bass as bass
import concourse.mybir as mybir
import concourse.tile as tile
from concourse import bacc
from concourse.bass_utils import run_bass_kernel_spmd

# Problem constants (nn_DT_GCN_Lite): hardcoded per harness contract.
N_NODES = 100000
N_EDGES = 1000000
IN_CH = 64
OUT_CH = 128
N_CORES = 8

WINDOW = 128                       # dest nodes per window
WINDOWS_PER_CORE = 98
NODES_PER_CORE = WINDOWS_PER_CORE * WINDOW   # 12544
N_NODES_PAD = NODES_PER_CORE * N_CORES       # 100352
N_WINDOWS = WINDOWS_PER_CORE * N_CORES       # 784

P = 128                            # edges per block (one partition each)
CHUNK = 25000                      # nodes per x-chunk (int16 gather idx)
N_CHUNKS = 4
G = 7                              # windows per gather group (98 = 14 * 7)
N_GROUPS = WINDOWS_PER_CORE // G
GEMM_G = 4                         # windows per output GEMM group
MAX_NI = 896                       # idx per gather (ring-limited)
DMA_SCRATCH = 16384

FP = mybir.dt.float32
BF = mybir.dt.bfloat16
NP_FP = np.float32
NP_BF = ml_dtypes.bfloat16


def _layout(cap_wc):
    """Static layout from cap_wc [W, C] (padded per-bucket edge counts,
    multiples of 128, shared across cores)."""
    W = cap_wc.shape[0]
    nblk_wc = cap_wc // P                               # [W, C]
    nblk_w = nblk_wc.sum(axis=1)

    groups = [list(range(g * G, (g + 1) * G)) for g in range(W // G)]
    # msg/block column order: (g, ch, w, j); gathers are per (g, ch)
    blk_of = {}            # (w, ch) -> absolute block offset of that bucket
    gth = []               # (gi, ch, ni, sidx_off, blk_off) per sub-gather
    nblk = 0
    sidx = 0
    for gi, ws in enumerate(groups):
        for ch in range(N_CHUNKS):
            ni = int(cap_wc[ws, ch].sum())
            done = 0
            while done < ni:
                sub = min(MAX_NI, ni - done)
                gth.append((gi, ch, sub, sidx, nblk + done // P))
                sidx += sub // 16
                done += sub
            for w in ws:
                blk_of[(w, ch)] = nblk
                nblk += int(nblk_wc[w, ch])
    # blocks of each window in matmul order (ch-major), as absolute indices
    wblocks = {
        w: [blk_of[(w, ch)] + j
            for ch in range(N_CHUNKS) for j in range(int(nblk_wc[w, ch]))]
        for w in range(W)
    }
    # msg columns per gather group (for tile sizing / group base offsets)
    gbase = [min(blk_of[(ws[0], ch)] for ch in range(N_CHUNKS)) for ws in groups]
    gcols = []
    for gi, ws in enumerate(groups):
        end = max(blk_of[(w, ch)] + int(nblk_wc[w, ch])
                  for w in ws for ch in range(N_CHUNKS))
        gcols.append(end - gbase[gi])
    return dict(
        nblk_wc=nblk_wc, nblk_w=nblk_w, NBLK=nblk, SIDX=sidx,
        groups=groups, blk_of=blk_of, wblocks=wblocks,
        gth=gth, gbase=gbase, gcols=gcols,
    )


def build_nc(cap_wc, repeat=1):
    L = _layout(cap_wc)
    NBLK, SIDX = L["NBLK"], L["SIDX"]
    max_gcols = max(L["gcols"])
    max_nblk_w = int(L["nblk_w"].max())

    nc = bacc.Bacc(
        "TRN2", target_bir_lowering=False, num_swdge_queues=4,
        dynamic_dma_scratch_size=DMA_SCRATCH,
    )

    x2_d = nc.dram_tensor("x2", [N_NODES, 2 * IN_CH], BF, kind="ExternalInput")
    idx_d = nc.dram_tensor("idx16", [P, SIDX], mybir.dt.int16, kind="ExternalInput")
    rowl_d = nc.dram_tensor("rowl", [P, NBLK], FP, kind="ExternalInput")
    wts_d = nc.dram_tensor("wts", [P, NBLK], FP, kind="ExternalInput")
    wt_d = nc.dram_tensor("wt", [IN_CH, OUT_CH], BF, kind="ExternalInput")
    out_d = nc.dram_tensor("out", [OUT_CH, NODES_PER_CORE], BF, kind="ExternalOutput")

    with tile.TileContext(nc) as tc:
        with (
            tc.tile_pool(name="const", bufs=1) as const_pool,
            tc.tile_pool(name="msg", bufs=3) as msg_pool,
            tc.tile_pool(name="oh", bufs=8) as oh_pool,
            tc.tile_pool(name="aggp", bufs=4, space="PSUM") as aggp_pool,
            tc.tile_pool(name="agg4", bufs=3) as agg4_pool,
            tc.tile_pool(name="outp", bufs=2, space="PSUM") as outp_pool,
            tc.tile_pool(name="outs", bufs=3) as outs_pool,
        ):
            idx_sb = const_pool.tile([P, SIDX], mybir.dt.int16)
            rowl_sb = const_pool.tile([P, NBLK], FP)
            wts_sb = const_pool.tile([P, NBLK], FP)
            iota_f = const_pool.tile([P, WINDOW], FP)
            iota_sb = const_pool.tile([P, WINDOW], BF)
            wt_sb = const_pool.tile([IN_CH, OUT_CH], BF)

            nc.sync.dma_start(idx_sb[:], idx_d[:])
            nc.sync.dma_start(rowl_sb[:], rowl_d[:])
            nc.sync.dma_start(wts_sb[:], wts_d[:])
            nc.sync.dma_start(wt_sb[:], wt_d[:])
            nc.gpsimd.iota(
                iota_f[:], pattern=[[1, WINDOW]], base=0,
                channel_multiplier=0, allow_small_or_imprecise_dtypes=True,
            )
            nc.vector.tensor_copy(iota_sb[:], iota_f[:])

            loop_cm = tc.For_i(0, repeat, 1) if repeat > 1 else contextlib.nullcontext()
            with loop_cm:
                agg4 = None
                for gi, ws in enumerate(L["groups"]):
                    gbase = L["gbase"][gi]
                    gcols = L["gcols"][gi]
                    msg = msg_pool.tile([P, max_gcols * 2 * IN_CH], BF, tag="msg")
                    for gord, (gi2, ch, ni, sidx_off, blk_off) in enumerate(L["gth"]):
                        if gi2 != gi or ni == 0:
                            continue
                        k = ni // P
                        mo = blk_off - gbase
                        nc.gpsimd.dma_gather(
                            out_ap=msg[:, mo * 2 * IN_CH : (mo + k) * 2 * IN_CH]
                            .rearrange("p (k d) -> p k d", k=k),
                            in_ap=x2_d[ch * CHUNK : min((ch + 1) * CHUNK, N_NODES), :],
                            idxs_ap=idx_sb[:, sidx_off : sidx_off + ni // 16],
                            num_idxs=ni,
                            num_idxs_reg=ni,
                            elem_size=2 * IN_CH,
                            queue_num=gord % 4,
                        )
                    for w in ws:
                        blocks = L["wblocks"][w]
                        nblkw = len(blocks)
                        if nblkw == 0:
                            continue
                        ohw = oh_pool.tile([P, max_nblk_w * WINDOW], BF, tag="oh")
                        for jj, b in enumerate(blocks):
                            # weighted one-hot: (iota == rowl[b]) * wts[b]
                            nc.vector.tensor_scalar(
                                out=ohw[:, jj * WINDOW : (jj + 1) * WINDOW],
                                in0=iota_sb[:],
                                scalar1=rowl_sb[:, b : b + 1],
                                scalar2=wts_sb[:, b : b + 1],
                                op0=mybir.AluOpType.is_equal,
                                op1=mybir.AluOpType.mult,
                            )
                        aggT = aggp_pool.tile([IN_CH, WINDOW], FP)
                        for jj, b in enumerate(blocks):
                            mc = (b - gbase) * 2 * IN_CH
                            nc.tensor.matmul(
                                aggT[:],
                                lhsT=msg[:, mc : mc + IN_CH],
                                rhs=ohw[:, jj * WINDOW : (jj + 1) * WINDOW],
                                start=(jj == 0),
                                stop=(jj == nblkw - 1),
                            )
                        gq, gr = divmod(w, GEMM_G)
                        if gr == 0:
                            agg4 = agg4_pool.tile([IN_CH, GEMM_G * WINDOW], BF, tag="agg4")
                        nc.scalar.copy(
                            agg4[:, gr * WINDOW : (gr + 1) * WINDOW], aggT[:]
                        )
                        if gr == GEMM_G - 1 or w == WINDOWS_PER_CORE - 1:
                            width = (gr + 1) * WINDOW
                            op = outp_pool.tile([OUT_CH, GEMM_G * WINDOW], FP)
                            nc.tensor.matmul(
                                op[:, :width],
                                lhsT=wt_sb[:],
                                rhs=agg4[:, :width],
                                start=True, stop=True,
                            )
                            outs = outs_pool.tile([OUT_CH, GEMM_G * WINDOW], BF)
                            nc.scalar.copy(outs[:, :width], op[:, :width])
                            nc.sync.dma_start(
                                out_d[:, gq * GEMM_G * WINDOW : gq * GEMM_G * WINDOW + width],
                                outs[:, :width],
                            )
    nc.compile()
    return nc


def preprocess(x, edge_index, edge_weight):
    """Bucket edges by (window, chunk); permute windows across (slot, core) to
    balance counts; pad buckets to multiples of P (shared across cores).
    Returns per-core input maps and cap_wc. The window permutation is stored
    in in_maps[0]['_perm'] (popped before launch)."""
    row = np.asarray(edge_index[0], dtype=np.int64)
    col = np.asarray(edge_index[1], dtype=np.int64)
    wts = np.asarray(edge_weight, dtype=NP_FP)

    gwin = row >> 7
    ch = col // CHUNK
    key = gwin * N_CHUNKS + ch
    order = np.argsort(key, kind="stable")
    row_s, col_s, w_s = row[order], col[order], wts[order]

    n_keys = N_WINDOWS * N_CHUNKS
    counts = np.bincount(key[order], minlength=n_keys).reshape(N_WINDOWS, N_CHUNKS)
    starts = np.zeros(n_keys + 1, dtype=np.int64)
    np.cumsum(counts.reshape(-1), out=starts[1:])

    # window permutation: sort by total count desc, rank i -> (slot i//8, core i%8)
    tot = counts.sum(axis=1)
    rank = np.argsort(-tot, kind="stable")
    perm = rank.reshape(WINDOWS_PER_CORE, N_CORES)      # [slot, core] -> gwin

    # shared caps: max over cores per (slot, chunk), padded to P
    cnt_sc = counts[perm]                               # [slot, core, chunk] -> wait
    # counts[perm] has shape [slot, core, chunk]
    cap_wc = -(-cnt_sc.max(axis=1) // P) * P            # [slot, chunk]
    for s in range(WINDOWS_PER_CORE):
        if cap_wc[s].sum() == 0:
            cap_wc[s, 0] = P

    L = _layout(cap_wc)
    NBLK, SIDX = L["NBLK"], L["SIDX"]
    nblk_wc = L["nblk_wc"]

    in_maps = []
    for c in range(N_CORES):
        rowl_a = np.zeros((P, NBLK), dtype=NP_FP)
        wts_a = np.zeros((P, NBLK), dtype=NP_FP)
        idx_blk = np.zeros((P, NBLK), dtype=np.int16)
        for s in range(WINDOWS_PER_CORE):
            g = int(perm[s, c])
            for chn in range(N_CHUNKS):
                nblk = int(nblk_wc[s, chn])
                if nblk == 0:
                    continue
                k = g * N_CHUNKS + chn
                st, e = starts[k], starts[k + 1]
                cnt = e - st
                cap = nblk * P
                b0 = L["blk_of"][(s, chn)]
                re_ = np.zeros((cap,), dtype=NP_FP)
                we = np.zeros((cap,), dtype=NP_FP)
                ce = np.zeros((cap,), dtype=np.int16)
                re_[:cnt] = (row_s[st:e] - g * WINDOW).astype(NP_FP)
                we[:cnt] = w_s[st:e]
                ce[:cnt] = (col_s[st:e] - chn * CHUNK).astype(np.int16)
                # edge i -> partition i%128, block i//128
                rowl_a[:, b0 : b0 + nblk] = re_.reshape(nblk, P).T
                wts_a[:, b0 : b0 + nblk] = we.reshape(nblk, P).T
                idx_blk[:, b0 : b0 + nblk] = ce.reshape(nblk, P).T

        # idx16: wrapped indices per sub-gather stream
        idx16 = np.zeros((P, SIDX), dtype=np.int16)
        for (gi, chn, ni, sidx_off, blk_off) in L["gth"]:
            if ni == 0:
                continue
            k = ni // P
            stream = idx_blk[:, blk_off : blk_off + k].T.reshape(-1)  # (blk, part)
            wrapped = stream.reshape(ni // 16, 16).T                  # [16, ni/16]
            idx16[:, sidx_off : sidx_off + ni // 16] = np.tile(wrapped, (8, 1))

        in_maps.append({"idx16": idx16, "rowl": rowl_a, "wts": wts_a})
    in_maps[0]["_perm"] = perm
    return in_maps, cap_wc


def finish_inputs(in_maps, inputs):
    """Attach replicated tensors (x2, wt) to each core's in_map."""
    in_maps[0].pop("_perm", None)
    x = np.asarray(inputs["x"], dtype=NP_FP)
    xb = x.astype(NP_BF)
    x2 = np.empty((N_NODES, 2 * IN_CH), dtype=NP_BF)
    x2[:, :IN_CH] = xb
    x2[:, IN_CH:] = xb
    wt = np.ascontiguousarray(np.asarray(inputs["W"], dtype=NP_FP).T).astype(NP_BF)
    for c in range(N_CORES):
        in_maps[c]["x2"] = x2
        in_maps[c]["wt"] = wt


_CACHE = {}


def kernel(x, edge_index, edge_weight, W, b):
    in_maps, cap_wc = preprocess(x, edge_index, edge_weight)
    perm = in_maps[0].pop("_perm")

    key = cap_wc.tobytes()
    if key not in _CACHE:
        _CACHE[key] = build_nc(cap_wc)
    nc = _CACHE[key]

    finish_inputs(in_maps, {"x": x, "W": W})

    res = run_bass_kernel_spmd(nc, in_maps, core_ids=list(range(N_CORES)))
    outT = np.stack([res.results[c]["out"] for c in range(N_CORES)])  # [8,128,12544]

    out_full = np.empty((N_NODES_PAD, OUT_CH), dtype=NP_FP)
    bb = np.asarray(b, dtype=NP_FP)
    for c in range(N_CORES):
        oc = outT[c].astype(NP_FP)                    # [128, 12544]
        for s in range(WINDOWS_PER_CORE):
            g = int(perm[s, c])
            out_full[g * WINDOW : (g + 1) * WINDOW, :] = oc[
                :, s * WINDOW : (s + 1) * WINDOW
            ].T
    return out_full[:N_NODES] + bb
